# revision 1
# baseline (speedup 1.0000x reference)
"""Clockwork RNN (CwRNNCell) Trainium2 Bass kernel.

Reference semantics (T=4096, H=2048, 8 modules of 256, periods 2^j):
  step t (1-indexed): module j active iff t % 2^j == 0
  pre = x_t @ W_ih.T + b_ih + h @ W_hh.T + b_hh
  h[active] = tanh(pre[active]);  inactive modules hold.

Design:
  Stage A (PE, fp32): U = X @ W_ih.T + (b_ih+b_hh), written to DRAM as a
    bf16 hi+lo pair in a (tau,f,p)-swizzled layout so the chain can pull it
    in with 16-bit transpose-DMAs (fp32 transpose-DMA is unsupported).
  Chain (single core, everything SBUF-resident): the serial recurrence is
    decomposed into per-module "events".  At step t the active set is the
    module prefix 0..nu (nu = ctz(t) capped at 7).  We cache per-input-module
    column products Q_k = W_hh[:, blk_k] @ h_k and refresh Q_k only when
    module k updates, covering exactly the contiguous row prefix R(k,t) of
    future readers.  pre = sum_k Q_k + U via one DVE reduce over a 9-slot
    interleaved layout (slot 8 holds U_t).  tanh on ScalarE -> bf16 h.
  Weights W_hh are bf16 (validated: 9.0e-3 absmax err vs fp32 reference,
    contractive dynamics, no drift).  PE weight-load bandwidth is the wall;
    event coverage is provably minimal for this caching scheme.
  Output rows are staged [p, 16*tau+f] per 128-step superstep, PE-transposed
    back to row-major, and DMA'd out.
"""

import numpy as np
import ml_dtypes

import concourse.bass as bass
import concourse.bacc as bacc
import concourse.mybir as mybir
from concourse import tile
from concourse.bass_utils import run_bass_kernel_spmd

BF16 = mybir.dt.bfloat16
F32 = mybir.dt.float32

H = 2048
IN = 1024
MS = 256
SL = 128  # superstep length (= max period)
NSLOT = 9  # 8 Q slots + 1 U slot


def _ctz(t):
    return (t & -t).bit_length() - 1


def _nu(t):
    return min(_ctz(t), 7)


def _cov(k, t):
    """#modules (contiguous prefix) covered by event (k, t). Downward closed."""
    imax = k
    for i in range(k + 1, 8):
        if (t % (1 << i)) >= (1 << i) - (1 << k):
            imax = i
        else:
            break
    return imax + 1


def build_nc(T=4096, num_cores=1, debug=False, enable_asserts=False, repeat=1):
    """repeat>1 re-runs the chain loop (same data, s mod SS addressing) for
    amortized timing: marginal cost per repeat = true chain exec time."""
    SS = T // SL
    nc = bacc.Bacc(
        "TRN2",
        target_bir_lowering=False,
        debug=debug,
        enable_asserts=enable_asserts,
        num_devices=num_cores,
    )

    xT = nc.dram_tensor("xT", [IN, T], F32, kind="ExternalInput").ap()
    wihT = nc.dram_tensor("wihT", [IN, H], F32, kind="ExternalInput").ap()
    whhT = nc.dram_tensor("whhT", [H, H], BF16, kind="ExternalInput").ap()
    bias_bc = nc.dram_tensor("bias_bc", [128, H], F32, kind="ExternalInput").ap()
    ident = nc.dram_tensor("ident", [128, 128], F32, kind="ExternalInput").ap()
    out = nc.dram_tensor("out", [T, H], F32, kind="ExternalOutput").ap()

    # U (hi/lo bf16) in swizzled layout: row (s*SL*16 + tau*16 + f), col p
    # holds U[t = s*SL + tau, 128*f + p].
    uhi = nc.dram_tensor("uhi", [T * 16, 128], BF16).ap()
    ulo = nc.dram_tensor("ulo", [T * 16, 128], BF16).ap()

    with tile.TileContext(nc) as tc:
        # ---------------- Stage A: U = X @ W_ih.T + bias ----------------
        with (
            tc.tile_pool(name="sa_w", bufs=1) as sa_w,
            tc.tile_pool(name="sa_x", bufs=3) as sa_x,
            tc.tile_pool(name="sa_u", bufs=4) as sa_u,
            tc.tile_pool(name="sa_ps", bufs=4, space="PSUM") as sa_ps,
        ):
            wih_sb = sa_w.tile([128, 8 * H], F32)  # 8 K-tiles of W_ih.T
            for c in range(8):
                nc.sync.dma_start(
                    wih_sb[:, c * H : (c + 1) * H], wihT[c * 128 : (c + 1) * 128, :]
                )
            bias_sb = sa_w.tile([128, H], F32)
            nc.sync.dma_start(bias_sb[:], bias_bc[:])

            uhi_v = uhi.rearrange("(t f) p -> t f p", f=16)
            ulo_v = ulo.rearrange("(t f) p -> t f p", f=16)

            for s in range(SS):
                xt_t = sa_x.tile([128, 8 * 128], F32, tag="xt")
                for c in range(8):
                    # lhsT tile: X.T[128c:128c+128, s*SL : s*SL+128]
                    nc.sync.dma_start(
                        xt_t[:, c * 128 : (c + 1) * 128],
                        xT[c * 128 : (c + 1) * 128, s * SL : (s + 1) * SL],
                    )
                for n in range(4):  # r-chunks of 512
                    ps = sa_ps.tile([128, 512], F32, tag="aps")
                    for c in range(8):
                        nc.tensor.matmul(
                            ps[:],
                            xt_t[:, c * 128 : (c + 1) * 128],
                            wih_sb[:, c * H + n * 512 : c * H + (n + 1) * 512],
                            start=(c == 0),
                            stop=(c == 7),
                        )
                    uf = sa_u.tile([128, 512], F32, tag="uf")
                    nc.vector.tensor_add(
                        uf[:], ps[:], bias_sb[:, n * 512 : (n + 1) * 512]
                    )
                    uh = sa_u.tile([128, 512], BF16, tag="uh")
                    nc.scalar.activation(
                        uh[:], uf[:], mybir.ActivationFunctionType.Copy
                    )
                    ul = sa_u.tile([128, 512], BF16, tag="ul")
                    nc.vector.tensor_sub(ul[:], uf[:], uh[:])
                    # store swizzled: [tau part, (f 4, p 128)] -> rows s*SL*16+tau*16+(4n+fi), col p
                    dst_h = uhi_v[s * SL * 16 // 16 : (s + 1) * SL * 16 // 16, 4 * n : 4 * n + 4, :]
                    dst_l = ulo_v[s * SL * 16 // 16 : (s + 1) * SL * 16 // 16, 4 * n : 4 * n + 4, :]
                    src_h = uh[:].rearrange("t (f p) -> t f p", p=128)
                    src_l = ul[:].rearrange("t (f p) -> t f p", p=128)
                    nc.sync.dma_start(dst_h, src_h)
                    nc.sync.dma_start(dst_l, src_l)

        # ---------------- Chain ----------------
        with (
            tc.tile_pool(name="ch_w", bufs=1) as ch_w,
            tc.tile_pool(name="ch_st", bufs=1) as ch_st,
            tc.tile_pool(name="ch_u", bufs=1) as ch_u,
            tc.tile_pool(name="ch_o", bufs=1) as ch_o,
            tc.tile_pool(name="ch_ps", bufs=4, space="PSUM") as ch_ps,
            tc.tile_pool(name="ch_p0", bufs=1, space="PSUM") as ch_p0,
            tc.tile_pool(name="ch_pt", bufs=2, space="PSUM") as ch_pt,
        ):
            whh_sb = ch_w.tile([128, 16 * H], BF16)  # [p, c*2048 + m*128 + j]
            for c in range(16):
                nc.sync.dma_start(
                    whh_sb[:, c * H : (c + 1) * H], whhT[c * 128 : (c + 1) * 128, :]
                )
            ident_sb = ch_w.tile([128, 128], F32)
            nc.sync.dma_start(ident_sb[:], ident[:])

            q_sb = ch_st.tile([128, 16 * NSLOT], F32)  # [p, f*9 + slot]
            hbf = ch_st.tile([128, 16], BF16)
            pre_sb = ch_st.tile([128, 16], F32)
            nc.vector.memset(q_sb[:], 0.0)
            nc.vector.memset(hbf[:], 0.0)
            # module-0 product lives in a psum ping-pong pair, read directly
            # by the next step's consume (module 0 updates every step).
            q0_ps_a = ch_p0.tile([128, 16], F32)
            q0_ps_b = ch_p0.tile([128, 16], F32)
            nc.vector.memset(q0_ps_a[:], 0.0)
            nc.vector.memset(q0_ps_b[:], 0.0)

            uhi_sb = ch_u.tile([128, SL * 16], BF16)
            ulo_sb = ch_u.tile([128, SL * 16], BF16)
            u_sb = ch_u.tile([128, SL * 16], F32)
            out_sb = ch_o.tile([128, SL * 16], F32)
            orow_sb = ch_o.tile([128, H], F32)

            q9 = q_sb[:].rearrange("p (f k) -> p f k", k=NSLOT)
            tmp_red = ch_st.tile([128, 16], F32)

            # per-step static schedule (identical across supersteps)
            sched = []
            for tau in range(SL):
                t = tau + 1
                nu = _nu(t)
                evs = [(k, _cov(k, t)) for k in range(nu + 1)]
                sched.append((nu, evs))

            with tc.For_i(
                0,
                SS * repeat,
                1,
                hint_engines=(
                    mybir.EngineType.PE,
                    mybir.EngineType.Activation,
                    mybir.EngineType.DVE,
                ),
            ) as s_raw:
                s = (s_raw % SS) if repeat > 1 else s_raw
                # load U superstep (transpose-DMA: [SL*16, 128] -> [128, SL*16])
                nc.sync.dma_start(
                    uhi_sb[:], uhi[bass.ds(s * (SL * 16), SL * 16), :], transpose=True
                )
                nc.sync.dma_start(
                    ulo_sb[:], ulo[bass.ds(s * (SL * 16), SL * 16), :], transpose=True
                )
                nc.vector.tensor_add(u_sb[:], uhi_sb[:], ulo_sb[:])

                # Deferred emission plan: event (k,t)'s module-i rows (past
                # the next step's consume width) are due at the reduce of
                # t_i = next multiple of 2^i after t.  Each chunk may be
                # emitted anywhere in [creation step, t_i - 2] (hbf[module k]
                # is stable until k's next update, which coverage puts after
                # every deadline).  Greedy: tightest window first, placed at
                # the least-loaded step, so big-consume deadline crunches
                # spread backwards.  Emission happens BEFORE each step's
                # critical matmuls (in-order PE dispatch: work after a waiting
                # instruction cannot fill its wait window).
                base_load = [0] * (SL + 1)
                chunks = []  # (tau_create, k, li, ilo, ihi, window_lo, window_hi)
                for tau in range(SL):
                    nu_, evs_ = sched[tau]
                    t_ = tau + 1
                    f_next_ = 2 * (_nu(t_ + 1) + 1)
                    for k, cv in evs_:
                        if k == 0:
                            base_load[tau] += 4 * cv
                            continue
                        i_prompt = min(f_next_ // 2, cv)
                        base_load[tau] += 4 * i_prompt
                        by_step = {}
                        for i in range(i_prompt, cv):
                            t_i = ((t_ // (1 << i)) + 1) * (1 << i)
                            by_step.setdefault(t_i, []).append(i)
                        for li, (t_i, rows) in enumerate(sorted(by_step.items())):
                            lo, hi = min(rows), max(rows) + 1
                            chunks.append(
                                [tau, k, li, lo, hi, tau, min(t_i - 2, SL)]
                            )
                plan = {}
                CAP = 10**9  # latest-fit == deadline-1 placement (best measured)
                for ch in sorted(chunks, key=lambda c: c[6] - c[5]):
                    tau_c, k, li, lo, hi, wlo, whi = ch
                    best = whi
                    for cand in range(whi, wlo - 1, -1):
                        if base_load[cand] + 4 * (hi - lo) <= CAP:
                            best = cand
                            break
                    base_load[best] += 4 * (hi - lo)
                    plan.setdefault(best, []).append((tau_c, k, li, lo, hi))
                prev_k0_tail = 0

                def _emit_mms_early(psq, k, mlo, mhi):
                    for m in range(mlo, mhi):
                        for half in range(2):
                            c = 2 * k + half
                            nc.tensor.matmul(
                                psq[:, m : m + 1],
                                whh_sb[:, c * H + m * 128 : c * H + (m + 1) * 128],
                                hbf[:, c : c + 1],
                                start=(half == 0),
                                stop=(half == 1),
                            )

                for tau in range(SL):
                    tail_prev = prev_k0_tail
                    nu, evs = sched[tau]
                    Fc = 2 * (nu + 1)
                    t = tau + 1
                    q0_prev = q0_ps_b if tau % 2 == 0 else q0_ps_a
                    q0_cur = q0_ps_a if tau % 2 == 0 else q0_ps_b
                    # U into slot 8 (only the consumed cols)
                    nc.vector.tensor_copy(
                        q9[:, 0:Fc, 8], u_sb[:, 16 * tau : 16 * tau + Fc]
                    )
                    # pre = sum of slots 1..8 + previous-step module-0 product.
                    # Tight lane: cols 0:2 (module 0) feed the next k0 matmuls;
                    # lazy lane: cols 2:F have a full step of slack.
                    nc.vector.reduce_sum(
                        tmp_red[:, 0:Fc], q9[:, 0:Fc, 1:9], axis=mybir.AxisListType.X
                    )
                    # lazy tail of the PREVIOUS k0 event (cols 2+): reads the
                    # OLD h0, so it must be emitted before this step's tanh
                    # overwrites hbf[:, 0:2] (WAR tracked by Tile); it runs in
                    # the PE wait window of the tight lane.
                    if tail_prev:
                        _emit_mms_early(q0_prev, 0, 2, 2 * tail_prev)
                    Ft = min(2, Fc)
                    nc.vector.tensor_add(
                        pre_sb[:, 0:Ft], tmp_red[:, 0:Ft], q0_prev[:, 0:Ft]
                    )
                    nc.scalar.activation(
                        hbf[:, 0:Ft], pre_sb[:, 0:Ft], mybir.ActivationFunctionType.Tanh
                    )

                    def _emit_mms(psq, k, mlo, mhi):
                        for m in range(mlo, mhi):
                            for half in range(2):
                                c = 2 * k + half
                                nc.tensor.matmul(
                                    psq[:, m : m + 1],
                                    whh_sb[:, c * H + m * 128 : c * H + (m + 1) * 128],
                                    hbf[:, c : c + 1],
                                    start=(half == 0),
                                    stop=(half == 1),
                                )

                    # (1) planned deferred work: fills the PE wait on this
                    # step's critical matmuls
                    for (tau_c, kk, li, ilo, ihi) in plan.pop(tau, ()):
                        psq2 = ch_ps.tile(
                            [128, 16], F32, tag="qev", name=f"psqL_{tau_c}_{kk}_{li}"
                        )
                        _emit_mms(psq2, kk, 2 * ilo, 2 * ihi)
                        nc.scalar.copy(
                            q9[:, 2 * ilo : 2 * ihi, kk], psq2[:, 2 * ilo : 2 * ihi]
                        )
                    # (2) critical: module-0 event cols 0:2 (the tight cycle)
                    f_next = 2 * (_nu(t + 1) + 1)
                    k0_cv = evs[0][1]
                    _emit_mms(q0_cur, 0, 0, min(2, 2 * k0_cv))
                    prev_k0_tail = k0_cv if 2 * k0_cv > 2 else 0
                    # (2c) lazy consume lane: cols 2:F of pre/tanh + output row
                    if Fc > 2:
                        nc.vector.tensor_add(
                            pre_sb[:, 2:Fc], tmp_red[:, 2:Fc], q0_prev[:, 2:Fc]
                        )
                        nc.scalar.activation(
                            hbf[:, 2:Fc], pre_sb[:, 2:Fc],
                            mybir.ActivationFunctionType.Tanh,
                        )
                    # stage full h into output (cast bf16->fp32)
                    nc.vector.tensor_copy(out_sb[:, 16 * tau : 16 * tau + 16], hbf[:])
                    # (3) k>=1 events: prompt cols feed the next step's reduce;
                    # later rows are scheduled at their deadline step
                    for k, cv in evs:
                        if k == 0:
                            continue
                        psq = ch_ps.tile([128, 16], F32, tag="qev", name=f"psq_{tau}_{k}")
                        i_prompt = min(f_next // 2, cv)
                        _emit_mms(psq, k, 0, 2 * i_prompt)
                        # prompt copy on DVE: keeps the ACT FIFO (tanh lane) short
                        nc.vector.tensor_copy(q9[:, 0 : 2 * i_prompt, k], psq[:, 0 : 2 * i_prompt])
                if prev_k0_tail:
                    # leftover lazy k0 tail from tau=127 (q0_cur of that step
                    # is q0_ps_b); consumed by the next iteration's lazy adds
                    for m in range(2, 2 * prev_k0_tail):
                        for half in range(2):
                            nc.tensor.matmul(
                                q0_ps_b[:, m : m + 1],
                                whh_sb[:, half * H + m * 128 : half * H + (m + 1) * 128],
                                hbf[:, half : half + 1],
                                start=(half == 0),
                                stop=(half == 1),
                            )
                for (tau_c, kk, li, ilo, ihi) in plan.pop(SL, ()):
                    psq2 = ch_ps.tile(
                        [128, 16], F32, tag="qev", name=f"psqT_{tau_c}_{kk}_{li}"
                    )
                    def _emit_tail(psq2=psq2, kk=kk, ilo=ilo, ihi=ihi):
                        for m in range(2 * ilo, 2 * ihi):
                            for half in range(2):
                                c = 2 * kk + half
                                nc.tensor.matmul(
                                    psq2[:, m : m + 1],
                                    whh_sb[:, c * H + m * 128 : c * H + (m + 1) * 128],
                                    hbf[:, c : c + 1],
                                    start=(half == 0),
                                    stop=(half == 1),
                                )
                    _emit_tail()
                    nc.scalar.copy(
                        q9[:, 2 * ilo : 2 * ihi, kk], psq2[:, 2 * ilo : 2 * ihi]
                    )
                assert not plan, f"unscheduled deferred work: {sorted(plan)}"
                # output: transpose [p, tau] -> [tau, p] per f, then DMA rows
                for f in range(16):
                    tps = ch_pt.tile([128, 128], F32, tag="otp")
                    nc.tensor.transpose(
                        tps[:],
                        out_sb[:].rearrange("p (t f) -> p f t", f=16)[:, f, :],
                        ident_sb[:],
                    )
                    nc.scalar.copy(orow_sb[:, f * 128 : (f + 1) * 128], tps[:])
                nc.sync.dma_start(out[bass.ds(s * SL, SL), :], orow_sb[:])

    nc.compile()
    return nc


def _prep_inputs(x, W_ih, W_hh, b_ih, b_hh):
    T = x.shape[0]
    return {
        "xT": np.ascontiguousarray(x.T).astype(np.float32),
        "wihT": np.ascontiguousarray(W_ih.T).astype(np.float32),
        "whhT": np.ascontiguousarray(W_hh.T).astype(ml_dtypes.bfloat16),
        "bias_bc": np.broadcast_to(
            (b_ih + b_hh).astype(np.float32), (128, H)
        ).copy(),
        "ident": np.eye(128, dtype=np.float32),
    }


_CACHE = {}


def _run(inputs, T=4096, trace=False):
    key = T
    if key not in _CACHE:
        _CACHE[key] = build_nc(T=T)
    nc = _CACHE[key]
    res = run_bass_kernel_spmd(nc, [inputs], [0], trace=trace)
    return res


def kernel(x, W_ih, W_hh, b_ih, b_hh):
    x = np.asarray(x, dtype=np.float32)
    T = x.shape[0]
    inputs = _prep_inputs(x, np.asarray(W_ih), np.asarray(W_hh), np.asarray(b_ih), np.asarray(b_hh))
    res = _run(inputs, T=T)
    return np.asarray(res.results[0]["out"], dtype=np.float32)



# revision 11
# speedup vs baseline: 1.9888x; 1.9888x over previous
"""Clockwork RNN (CwRNNCell) Trainium2 Bass kernel.

Reference semantics (T=4096, H=2048, 8 modules of 256, periods 2^j):
  step t (1-indexed): module j active iff t % 2^j == 0
  pre = x_t @ W_ih.T + b_ih + h @ W_hh.T + b_hh
  h[active] = tanh(pre[active]);  inactive modules hold.

Design (single core, everything SBUF/PSUM-resident):
  Stage A (PE, fp32): U = X @ W_ih.T + (b_ih+b_hh), written to DRAM as a
    bf16 hi+lo pair in a (tau,f,p)-swizzled layout so the chain can pull it
    in with 16-bit transpose-DMAs (fp32 transpose-DMA is unsupported).
  Chain: per-step pre-activation tiles live in PSUM (4 banks per 128-step
    superstep, 32 steps x 16 cols per bank) and are built entirely by PE
    accumulation (per-element has_written semantics):
      - an fp32 identity matmul per bank deposits U (start=True opens the
        bank, everything after accumulates),
      - when module m updates at step tau (h_m := tanh output), its column
        product W_hh[:, m-half] @ h_m is deposited into ALL steps of the
        window (tau, tau+2^m] in one broadcast-rhs matmul per output
        row-half (stride-16 psum columns, N = window length),
      - module 0's product (fresh every step) lands as N=1 matmuls into the
        next step's active columns only.
    Consume is a single ScalarE tanh per step (PSUM -> bf16 h in SBUF); DVE
    only stages h into the output buffer (off the critical path).  No DVE
    reduce, no q-slot copies: the serial cycle is tanh -> k0 matmuls -> tanh
    (~0.4-0.6us/step on HW).
  Weights W_hh are bf16 (validated: ~9e-3 absmax err vs fp32 reference).
  U prefetch (transpose-DMA + hi/lo add) and the output path (PE transpose,
  DVE copy, DMA) are double-buffered across supersteps; the hardware loop
  body covers TWO supersteps so ping-pong parity stays static.
"""

import numpy as np
import ml_dtypes

import concourse.bass as bass
import concourse.bacc as bacc
import concourse.mybir as mybir
from concourse import tile
from concourse.bass_utils import run_bass_kernel_spmd

BF16 = mybir.dt.bfloat16
F32 = mybir.dt.float32

H = 2048
IN = 1024
MS = 256
SL = 128  # superstep length (= max period)
BK = 32   # steps per psum bank (512 fp32 cols / 16)


def _ctz(t):
    return (t & -t).bit_length() - 1


def _nu(t):
    return min(_ctz(t), 7)


def _fc(t):
    return 2 * (_nu(t) + 1)


def build_nc(T=4096, num_cores=1, debug=False, enable_asserts=False, repeat=1):
    SS = T // SL
    assert (SS * repeat) % 2 == 0
    nc = bacc.Bacc(
        "TRN2",
        target_bir_lowering=False,
        debug=debug,
        enable_asserts=enable_asserts,
        num_devices=num_cores,
    )

    xT = nc.dram_tensor("xT", [IN, T], F32, kind="ExternalInput").ap()
    wihT = nc.dram_tensor("wihT", [IN, H], F32, kind="ExternalInput").ap()
    whhT = nc.dram_tensor("whhT", [H, H], BF16, kind="ExternalInput").ap()
    bias_bc = nc.dram_tensor("bias_bc", [128, H], F32, kind="ExternalInput").ap()
    ident = nc.dram_tensor("ident", [128, 128], F32, kind="ExternalInput").ap()
    out = nc.dram_tensor("out", [T, H], F32, kind="ExternalOutput").ap()

    # U (hi/lo bf16) in swizzled layout: row (t*16 + f), col p holds
    # U[t, 128*f + p].  One extra superstep of rows so the steady-state
    # prefetch of s+1 needs no modulo wraparound (overrun read unused).
    uhi = nc.dram_tensor("uhi", [(T + SL) * 16, 128], BF16).ap()
    ulo = nc.dram_tensor("ulo", [(T + SL) * 16, 128], BF16).ap()

    # static per-step schedule
    # rh_max over a window (tau, tau+2^m]: widest consumer step
    def _rhmax(tau, m):
        return max(_fc(tp) for tp in range(tau + 1, tau + (1 << m) + 1))

    with tile.TileContext(nc) as tc:
        # ---------------- Stage A: U = X @ W_ih.T + bias ----------------
        with (
            tc.tile_pool(name="sa_w", bufs=1) as sa_w,
            tc.tile_pool(name="sa_x", bufs=3) as sa_x,
            tc.tile_pool(name="sa_u", bufs=4) as sa_u,
            tc.tile_pool(name="sa_ps", bufs=4, space="PSUM") as sa_ps,
        ):
            wih_sb = sa_w.tile([128, 8 * H], F32)  # 8 K-tiles of W_ih.T
            for c in range(8):
                nc.sync.dma_start(
                    wih_sb[:, c * H : (c + 1) * H], wihT[c * 128 : (c + 1) * 128, :]
                )
            bias_sb = sa_w.tile([128, H], F32)
            nc.sync.dma_start(bias_sb[:], bias_bc[:])

            uhi_v = uhi.rearrange("(t f) p -> t f p", f=16)
            ulo_v = ulo.rearrange("(t f) p -> t f p", f=16)

            for s in range(SS):
                xt_t = sa_x.tile([128, 8 * 128], F32, tag="xt")
                for c in range(8):
                    nc.sync.dma_start(
                        xt_t[:, c * 128 : (c + 1) * 128],
                        xT[c * 128 : (c + 1) * 128, s * SL : (s + 1) * SL],
                    )
                for n in range(4):  # r-chunks of 512
                    ps = sa_ps.tile([128, 512], F32, tag="aps")
                    for c in range(8):
                        nc.tensor.matmul(
                            ps[:],
                            xt_t[:, c * 128 : (c + 1) * 128],
                            wih_sb[:, c * H + n * 512 : c * H + (n + 1) * 512],
                            start=(c == 0),
                            stop=(c == 7),
                        )
                    uf = sa_u.tile([128, 512], F32, tag="uf")
                    nc.vector.tensor_add(
                        uf[:], ps[:], bias_sb[:, n * 512 : (n + 1) * 512]
                    )
                    uh = sa_u.tile([128, 512], BF16, tag="uh")
                    nc.scalar.activation(
                        uh[:], uf[:], mybir.ActivationFunctionType.Copy
                    )
                    ul = sa_u.tile([128, 512], BF16, tag="ul")
                    nc.vector.tensor_sub(ul[:], uf[:], uh[:])
                    dst_h = uhi_v[s * SL : (s + 1) * SL, 4 * n : 4 * n + 4, :]
                    dst_l = ulo_v[s * SL : (s + 1) * SL, 4 * n : 4 * n + 4, :]
                    src_h = uh[:].rearrange("t (f p) -> t f p", p=128)
                    src_l = ul[:].rearrange("t (f p) -> t f p", p=128)
                    nc.sync.dma_start(dst_h, src_h)
                    nc.sync.dma_start(dst_l, src_l)
                    if s == SS - 1:
                        # fill the prefetch-overrun pad with finite data
                        nc.sync.dma_start(
                            uhi_v[(s + 1) * SL : (s + 2) * SL, 4 * n : 4 * n + 4, :],
                            src_h,
                        )
                        nc.sync.dma_start(
                            ulo_v[(s + 1) * SL : (s + 2) * SL, 4 * n : 4 * n + 4, :],
                            src_l,
                        )

        # ---------------- Chain ----------------
        with (
            tc.tile_pool(name="ch_w", bufs=1) as ch_w,
            tc.tile_pool(name="ch_st", bufs=1) as ch_st,
            tc.tile_pool(name="ch_u", bufs=1) as ch_u,
            tc.tile_pool(name="ch_o", bufs=1) as ch_o,
            tc.tile_pool(name="ch_pre", bufs=4, space="PSUM") as ch_pre,
            tc.tile_pool(name="ch_pt", bufs=2, space="PSUM") as ch_pt,
        ):
            whh_sb = ch_w.tile([128, 16 * H], BF16)  # [q, c*2048 + rh*128 + p]
            for c in range(16):
                nc.sync.dma_start(
                    whh_sb[:, c * H : (c + 1) * H], whhT[c * 128 : (c + 1) * 128, :]
                )
            ident_sb = ch_w.tile([128, 128], F32)
            nc.sync.dma_start(ident_sb[:], ident[:])

            hbf = ch_st.tile([128, 16], BF16)
            nc.vector.memset(hbf[:], 0.0)

            # explicit ping-pong buffers (static parity inside the HW loop)
            ubufs = []
            for k in range(2):
                ubufs.append((
                    ch_u.tile([128, SL * 16], BF16, name=f"uhi{k}"),
                    ch_u.tile([128, SL * 16], BF16, name=f"ulo{k}"),
                    ch_u.tile([128, SL * 16], F32, name=f"u{k}"),
                ))
            obufs = []
            for k in range(2):
                obufs.append((
                    ch_o.tile([128, SL * 16], F32, name=f"osb{k}"),
                    ch_o.tile([128, H], F32, name=f"orow{k}"),
                ))

            def wtile(c, rh):
                return whh_sb[:, c * H + rh * 128 : c * H + (rh + 1) * 128]

            def prefetch_u_dma(k, pf):
                """Transpose-DMA superstep pf's U hi/lo into ubufs[k]."""
                uh, ul, _ = ubufs[k]
                nc.sync.dma_start(
                    uh[:], uhi[bass.ds(pf * (SL * 16), SL * 16), :], transpose=True
                )
                nc.sync.dma_start(
                    ul[:], ulo[bass.ds(pf * (SL * 16), SL * 16), :], transpose=True
                )

            def prefetch_u_add(k):
                uh, ul, uu = ubufs[k]
                nc.vector.tensor_add(uu[:], uh[:], ul[:])

            # Static emission schedule: every W_hh deposit is a chunk
            # (m, tau0, rh, c): module m updated at step tau0 (0 = superstep
            # carry-in), output row-half rh, contraction half c.  Its first
            # reader is tanh(tau0 + 2^(rh//2)) (or the single j-read in the
            # window for rh//2 > m), so the chunk is emitted in the PE stream
            # at slot deadline-1 — after that slot's tanh, ahead of the next
            # step's critical module-0 matmuls.  Slot 0 = before the step loop.
            emit_at = [[] for _ in range(SL)]  # slot tau: emitted after tanh(tau)
            for tau0 in range(0, SL):
                mmax = 7 if tau0 == 0 else _nu(tau0)
                m_lo = 0 if tau0 == 0 else 1
                for m in range(m_lo, mmax + 1):
                    rhm = _rhmax(tau0, m)
                    for rh in range(rhm):
                        j = rh // 2
                        if j <= m:
                            deadline = tau0 + (1 << j)
                        else:
                            p = 1 << j
                            deadline = ((tau0 // p) + 1) * p
                            if deadline > tau0 + (1 << m):
                                continue  # no j-read in window
                        for c in (2 * m, 2 * m + 1):
                            emit_at[deadline - 1].append((m, tau0, rh, c))

            def emit_superstep(k, s_expr, pf_expr, prefetch_next):
                """One 128-step superstep; ubufs[k] holds this superstep's U."""
                uu = ubufs[k][2]
                osb, orow = obufs[k]

                pre = [
                    ch_pre.tile([128, 512], F32, tag="pre", name=f"pre{k}_{b}")
                    for b in range(4)
                ]
                # U deposit opens each bank (overwrite), fp32 identity matmul
                for b in range(4):
                    # bank col layout is rh-major: col = rh*BK + (tau-1)%BK,
                    # u_sb is step-major: view as [p, f, t]
                    u_v = uu[:, 512 * b : 512 * (b + 1)].rearrange(
                        "p (t f) -> p f t", f=16
                    )
                    nc.tensor.matmul(
                        pre[b][:],
                        ident_sb[:],
                        u_v,
                        start=True,
                        stop=False,
                        skip_group_check=True,
                    )

                def emit_chunk(m, tau0, rh, c):
                    """Broadcast W.T[c-block, rh-block] @ h[c] into psum cols
                    {16*(tp-1)+rh : tp in (tau0, tau0+2^m]}, split per bank."""
                    n_w = 1 << m
                    tp0 = tau0 + 1
                    while tp0 <= tau0 + n_w:
                        b = (tp0 - 1) // BK
                        tp1 = min(tau0 + n_w, (b + 1) * BK)
                        n = tp1 - tp0 + 1
                        lt = (tp0 - 1) % BK
                        dst = pre[b][:, rh * BK + lt : rh * BK + lt + n]
                        nc.tensor.matmul(
                            dst,
                            wtile(c, rh),
                            hbf[:, c : c + 1].broadcast_to([128, n]),
                            start=False,
                            stop=False,
                            skip_group_check=True,
                        )
                        tp0 = tp1 + 1

                # slot 0: carry-in chunks due before tanh(1)
                for (m, tau0, rh, c) in emit_at[0]:
                    emit_chunk(m, tau0, rh, c)

                # step loop
                for tau in range(1, SL + 1):
                    b = (tau - 1) // BK
                    lc = 16 * ((tau - 1) % BK)
                    fc = _fc(tau)
                    # module-0 deposit for THIS step (h_0 from tanh(tau-1))
                    lt = (tau - 1) % BK
                    if tau > 1:
                        for rh in range(fc):
                            for c in (0, 1):
                                nc.tensor.matmul(
                                    pre[b][:, rh * BK + lt : rh * BK + lt + 1],
                                    wtile(c, rh),
                                    hbf[:, c : c + 1],
                                    start=False,
                                    stop=False,
                                    skip_group_check=True,
                                )
                    nc.scalar.activation(
                        hbf[:, 0:fc],
                        pre[b][:].rearrange("p (f t) -> p t f", t=BK)[:, lt, 0:fc],
                        mybir.ActivationFunctionType.Tanh,
                    )
                    nc.vector.tensor_copy(osb[:, lc + 512 * b : lc + 512 * b + 16], hbf[:])
                    # deferred deposit chunks due at this slot
                    if tau < SL:
                        for (m, tau0, rh, c) in emit_at[tau]:
                            emit_chunk(m, tau0, rh, c)
                    if tau == 16 and prefetch_next:
                        prefetch_u_dma(1 - k, pf_expr)
                    if tau == 48 and prefetch_next:
                        prefetch_u_add(1 - k)

                # output: transpose [p, tau] -> [tau, p] per f, then DMA rows
                for f in range(16):
                    tps = ch_pt.tile([128, 128], F32, tag="otp", name=f"otp{k}_{f}")
                    nc.tensor.transpose(
                        tps[:],
                        osb[:].rearrange("p (t f) -> p f t", f=16)[:, f, :],
                        ident_sb[:],
                    )
                    nc.vector.tensor_copy(orow[:, f * 128 : (f + 1) * 128], tps[:])
                nc.sync.dma_start(out[bass.ds(s_expr * SL, SL), :], orow[:])

            # initial prefetch: superstep 0 into ubufs[0]
            prefetch_u_dma(0, 0)
            prefetch_u_add(0)

            with tc.For_i(
                0,
                SS * repeat // 2,
                1,
                hint_engines=(
                    mybir.EngineType.PE,
                    mybir.EngineType.Activation,
                    mybir.EngineType.DVE,
                ),
            ) as i_raw:
                for k in (0, 1):
                    if repeat == 1:
                        s_expr = 2 * i_raw + k
                        pf_expr = 2 * i_raw + k + 1
                    else:
                        s_expr = (2 * i_raw + k) % SS
                        pf_expr = (2 * i_raw + k + 1) % SS
                    emit_superstep(k, s_expr, pf_expr, prefetch_next=True)

    nc.compile()
    return nc


def _prep_inputs(x, W_ih, W_hh, b_ih, b_hh):
    return {
        "xT": np.ascontiguousarray(x.T).astype(np.float32),
        "wihT": np.ascontiguousarray(W_ih.T).astype(np.float32),
        "whhT": np.ascontiguousarray(W_hh.T).astype(ml_dtypes.bfloat16),
        "bias_bc": np.broadcast_to(
            (b_ih + b_hh).astype(np.float32), (128, H)
        ).copy(),
        "ident": np.eye(128, dtype=np.float32),
    }


_CACHE = {}


def _run(inputs, T=4096, trace=False):
    key = T
    if key not in _CACHE:
        _CACHE[key] = build_nc(T=T)
    nc = _CACHE[key]
    res = run_bass_kernel_spmd(nc, [inputs], [0], trace=trace)
    return res


def kernel(x, W_ih, W_hh, b_ih, b_hh):
    x = np.asarray(x, dtype=np.float32)
    T = x.shape[0]
    inputs = _prep_inputs(x, np.asarray(W_ih), np.asarray(W_hh), np.asarray(b_ih), np.asarray(b_hh))
    res = _run(inputs, T=T)
    return np.asarray(res.results[0]["out"], dtype=np.float32)
